# revision 64
# baseline (speedup 1.0000x reference)
"""Trainium2 Bass kernel for char-CNN: 5-tap conv along word_length + max-pool.

Reference computation (per (batch, sentence) word, shapes B=64 S=256 W=20 E=128):
    y[w, e] = sum_{kh=0..4} x[w + kh - 2, e] * conv_w[kh]     (zero padded)
    out[e]  = max_w y[w, e] + conv_b

Strategy:
  - Data-parallel over 8 NeuronCores: 8 batches (2048 words) per core.
  - Host pre-arranges each core's shard to z[(j w)=120, group=342, e=128]
    (groups of J=6 words, last group zero-padded) so every DMA descriptor
    is a multi-KiB contiguous run per partition — full HBM bandwidth.
  - The conv is a banded 20x20 matrix applied per word, done on TensorE:
    stationary lhsT = x6 [K=120 (6 words x 20 w_in), M=128 (e)], moving
    rhs = block-diagonal A [120, 120] -> PSUM [128 (e), 120 (6w x 20 w_out)].
    fp16 operands (1 cycle/row on PE; fp32 would be 4).
  - Max over w_out is a free-dim reduce on VectorE straight out of PSUM:
    [128, (groups, 20)] -> [128, groups*6] into a persistent [128, NW]
    maxima tile; one DMA out at the end (host transposes back).
  - Input DMAs are spread across the SP-HWDGE / ACT-HWDGE / SWDGE rings so
    the 16 SDMA engines always have in-flight work (one FIFO ring alone
    leaves completion-latency bubbles).  The SWDGE (gpsimd) ring casts
    f32 -> f16 in the DMA datapath; HWDGE rings land f32 and ScalarE casts.
"""

from contextlib import ExitStack

import numpy as np

import concourse.bass as bass
import concourse.mybir as mybir
import concourse.tile as tile
from concourse import bacc

W = 20  # word length
E = 128  # embedding dim
KH = 5  # conv taps
PAD = 2
J = 6  # words per matmul group (6 * 20 = 120 <= 128 partitions)
KP = J * W  # contraction size / partitions used (120)
CG = 16  # groups per compute sub-chunk (4 PSUM banks)
NCORES = 8
BANK = 512  # PSUM bank size in f32 elements


def build_conv_matrix(conv_w: np.ndarray) -> np.ndarray:
    """[KP, KP] conv matrix, output columns ordered w_out-major:
    A[j*W + wi, wo*J + j] = conv_w[wi - wo + 2].  The w-major column
    order makes the PSUM output planar so every max level on VectorE is
    a contiguous step-1 f16 tensor_max (2x mode)."""
    wv = np.asarray(conv_w, np.float32).reshape(-1)
    assert wv.shape == (KH,)
    a = np.zeros((KP, KP), np.float32)
    for j in range(J):
        for wo in range(W):
            for kh in range(KH):
                wi = wo + kh - PAD
                if 0 <= wi < W:
                    a[j * W + wi, wo * J + j] = wv[kh]
    return a.astype(np.float16)


def pack_input(x_core: np.ndarray, ng: int) -> np.ndarray:
    """[nw, W, E] f32 -> [KP, ng, E] f16 partition-major, zero-padded to
    ng*J words. The fp16 cast is the same one the kernel's compute path
    uses (TensorE consumes fp16); doing it host-side halves HBM traffic."""
    nw = x_core.shape[0]
    xp = np.zeros((ng * J, W, E), np.float16)
    xp[:nw] = x_core.astype(np.float16)
    # (g j) w e -> (j w) g e
    return np.ascontiguousarray(
        xp.reshape(ng, J, W, E).transpose(1, 2, 0, 3).reshape(KP, ng, E)
    )


def chunk_plan(ng: int, big: int = 64) -> list[int]:
    """Descending chunk sizes: big early (fewer ring bubbles while the
    stream is deep), small at the end (short pipeline tail)."""
    sizes = []
    rem = ng
    for sz, keep in ((64, 96), (32, 48), (16, 24), (8, 8)):
        if sz > big:
            continue
        while rem >= max(sz, keep):
            sizes.append(sz)
            rem -= sz
    if rem:
        sizes.append(rem)
    return sizes


def build_nc(
    nw: int,
    dma_rings: tuple[str, ...] = ("gpsimd",),
    bufs: int = 24,
    first_ring: str | None = None,
    big_chunk: int = 16,
    cg: int = 16,
    psum_bufs: int = 4,
    d_every: int = 12,
    deep_spool: int = 8,
) -> bass.Bass:
    """Build the per-core Bass graph. nw = real words per core.

    dma_rings: which descriptor rings carry the input stream, round-robin
    per chunk. 'gpsimd' (SWDGE) casts f32->f16 during the DMA; HWDGE rings
    ('sync'/'scalar') land f32 and ScalarE casts to f16.
    """
    f32 = mybir.dt.float32
    f16 = mybir.dt.float16
    ng = (nw + J - 1) // J  # padded group count
    nwp = ng * J  # padded word count

    nc = bacc.Bacc()
    z_ext = nc.declare_dram_parameter("z", [KP, ng, E], f16, isOutput=False)
    a_ext = nc.declare_dram_parameter("a", [KP, KP], f16, isOutput=False)
    out_ext = nc.declare_dram_parameter("out", [E, nw], f32, isOutput=True)

    engines = {
        "sync": nc.sync,
        "scalar": nc.scalar,
        "gpsimd": nc.gpsimd,
    }

    with ExitStack() as ctx:
        tc = ctx.enter_context(tile.TileContext(nc))
        const = ctx.enter_context(tc.tile_pool(name="const", bufs=1))
        hpool = ctx.enter_context(tc.tile_pool(name="xh", bufs=bufs))
        opool = ctx.enter_context(tc.tile_pool(name="o", bufs=1))
        spool = ctx.enter_context(tc.tile_pool(name="ys", bufs=deep_spool))
        t1pool = ctx.enter_context(tc.tile_pool(name="t1", bufs=deep_spool))
        u2pool = ctx.enter_context(tc.tile_pool(name="u2", bufs=deep_spool))
        u3pool = ctx.enter_context(tc.tile_pool(name="u3", bufs=deep_spool))
        u4pool = ctx.enter_context(tc.tile_pool(name="u4", bufs=deep_spool))
        pspool = ctx.enter_context(
            tc.tile_pool(name="ps", bufs=psum_bufs, space="PSUM")
        )
        ps_banks = (cg + 3) // 4  # PSUM banks per compute sub-chunk

        a_t = const.tile([KP, KP], f16)
        nc.sync.dma_start(out=a_t[:, :], in_=a_ext[:, :])
        maxt = opool.tile([E, nwp], f32)

        HW = W // 2  # 10

        def do_matmuls(xh, coff, sn):
            """Fill 2-bank PSUM tiles (finer slot rotation than one big
            4-bank tile). Returns [(tile, ngroups), ...] of <=8 groups."""
            tiles = []
            for h in range(0, sn, 8):
                hn = min(8, sn - h)
                ps = pspool.tile([E, 2 * BANK], f32, tag="ps")
                for g2 in range(hn):
                    g = h + g2
                    col = (g2 // 4) * BANK + (g2 % 4) * KP
                    nc.tensor.matmul(
                        ps[:, col : col + KP],
                        lhsT=xh[:, coff + g * E : coff + (g + 1) * E],
                        rhs=a_t[:, :],
                        start=True,
                        stop=True,
                    )
                tiles.append((ps, hn))
            return tiles

        def psum_view(ps, sn):
            """[E, nbank, c, W] view of sn (multiple of 4) groups."""
            nbank = sn // 4
            return (
                ps[:, 0 : nbank * BANK]
                .rearrange("p (k x) -> p k x", k=nbank)[:, :, 0 : 4 * J * W]
                .rearrange("p k (c w) -> p k c w", w=W)
            )

        def stage12_act(xh, coff, sg0, sn):
            """ACT parks the 20-block in SBUF as f16 in a TRANSPOSED planar
            layout s[w][word] (free: ACT is 1x regardless of write stride).
            Every max level is then a contiguous step-1 f16 tensor_max in
            DVE 2x mode: 20 -> 10 -> 5 -> (2,2,1) -> 1.  sn % 4 == 0."""
            tiles = do_matmuls(xh, coff, sn)
            # layout-preserving copies: psum cols are already (g, w, j)
            s = spool.tile([E, cg * J * W], f16, tag="ys")
            soff = 0
            for ps, hn in tiles:
                kb = hn // 4
                pin = ps[:, 0 : kb * BANK].rearrange(
                    "p (k x) -> p k x", k=kb
                )[:, :, 0 : 4 * J * W]
                sv = s[:, soff : soff + hn * J * W].rearrange(
                    "p (k x) -> p k x", k=kb
                )
                nc.scalar.copy(sv, pin)
                soff += hn * J * W
            # max tree: every level is max over the w axis of (g, w, j)
            # blocks — contiguous J-element runs, f16 2x mode throughout
            def tview(tile, nw_, w0, w1):
                return tile[:, 0 : sn * nw_ * J].rearrange(
                    "p (g w j) -> p g w j", g=sn, j=J
                )[:, :, w0:w1, :]

            u1 = t1pool.tile([E, cg * J * HW], f16, tag="t1")
            nc.vector.tensor_max(
                u1[:, 0 : sn * HW * J], tview(s, W, 0, HW), tview(s, W, HW, W)
            )
            u2 = u2pool.tile([E, cg * J * 5], f16, tag="u2")
            nc.vector.tensor_max(
                u2[:, 0 : sn * 5 * J], tview(u1, HW, 0, 5), tview(u1, HW, 5, 10)
            )
            u3 = u3pool.tile([E, cg * J * 2], f16, tag="u3")
            nc.vector.tensor_max(
                u3[:, 0 : sn * 2 * J], tview(u2, 5, 0, 2), tview(u2, 5, 2, 4)
            )
            u4 = u4pool.tile([E, cg * J], f16, tag="u4")
            nc.vector.tensor_max(
                u4[:, 0 : sn * J], tview(u3, 2, 0, 1), tview(u3, 2, 1, 2)
            )
            # final merge with the leftover 5th w-plane, f32 out to maxt
            nc.vector.tensor_max(
                maxt[:, sg0 * J : (sg0 + sn) * J].rearrange(
                    "p (g w j) -> p g w j", g=sn, j=J
                ),
                tview(u4, 1, 0, 1),
                tview(u2, 5, 4, 5),
            )
            return ("a", None, sg0, sn)

        def stage12_direct(xh, coff, sg0, sn):
            tiles = do_matmuls(xh, coff, sn)
            return ("d", tiles, sg0, sn)

        def stage3(kind, t, sg0, sn):
            if kind == "a":
                return  # the 'a' pipeline already wrote maxt
            # direct: reduce over w straight out of PSUM. Columns are
            # (g, w, j) per bank, so put w innermost in the AP.
            wcur = sg0 * J
            for ps, hn in t:
                for b in range((hn + 3) // 4):
                    gb = min(4, hn - 4 * b)
                    pv = ps[:, BANK * b : BANK * b + gb * J * W].rearrange(
                        "p (g w j) -> p g j w", g=gb, j=J
                    )
                    out_v = maxt[:, wcur : wcur + gb * J].rearrange(
                        "p (g j) -> p g j", g=gb
                    )
                    nc.vector.reduce_max(out_v, pv, axis=mybir.AxisListType.X)
                    wcur += gb * J

        g0 = 0
        if first_ring is not None:
            sizes = [16] + chunk_plan(ng - 16, big_chunk)
            rings = [first_ring] + [
                dma_rings[i % len(dma_rings)] for i in range(len(sizes) - 1)
            ]
        else:
            if ng > 32:
                # two small warm-up chunks so compute starts sooner
                sizes = [8, 8] + chunk_plan(ng - 16, big_chunk)
            else:
                sizes = chunk_plan(ng, big_chunk)
            rings = [dma_rings[i % len(dma_rings)] for i in range(len(sizes))]
        max_gn = max(sizes)

        # Phase A: the whole input stream is issued up front (bufs covers
        # every chunk) so no compute op can head-of-line-block a DMA
        # trigger on the gpsimd FIFO.
        subs = []
        for ring, gn in enumerate(sizes):
            eng_name = rings[ring]
            src = z_ext[:, g0 : g0 + gn, :].rearrange("p g e -> p (g e)")
            xh = hpool.tile([KP, max_gn * E], f16, tag="xh")
            engines[eng_name].dma_start(out=xh[:, 0 : gn * E], in_=src)
            for s0 in range(0, gn, cg):
                sn = min(cg, gn - s0)
                subs.append((xh, s0 * E, g0 + s0, sn))
            g0 += gn

        # Phase B: compute pipeline. stage3 follows its stage12 directly:
        # its dependency is the immediately preceding same-engine op (the
        # DVE tensor_max for ACT-path subs, the PE matmuls for direct).
        w_flushed = 0

        def flush_out(upto_words):
            nonlocal w_flushed
            hi = min(upto_words, nw)
            if hi - w_flushed >= 192 or (hi >= nw and hi > w_flushed):
                nc.sync.dma_start(
                    out=out_ext[:, w_flushed:hi], in_=maxt[:, w_flushed:hi]
                )
                w_flushed = hi

        for idx, sub in enumerate(subs):
            _, _, _, sn = sub
            if sn % 4 == 0 and (d_every == 0 or idx % d_every != d_every - 1):
                kind, t, sg0, sn = stage12_act(*sub)
            else:
                kind, t, sg0, sn = stage12_direct(*sub)
            stage3(kind, t, sg0, sn)
            flush_out(sg0 * J + sn * J)
    nc.finalize()
    return nc


def kernel(embedded_char, conv_w, conv_b):
    from concourse.bass_utils import run_bass_kernel_spmd

    x = np.asarray(embedded_char, np.float32)
    b_val = float(np.asarray(conv_b, np.float32).reshape(-1)[0])
    B, S, Wl, El = x.shape
    assert (Wl, El) == (W, E)
    bs = B // NCORES
    nw = bs * S
    ng = (nw + J - 1) // J
    a16 = build_conv_matrix(conv_w)

    nc = build_nc(nw)
    in_maps = [
        {
            "z": pack_input(x[i * bs : (i + 1) * bs].reshape(nw, Wl, El), ng),
            "a": a16,
        }
        for i in range(NCORES)
    ]
    res = run_bass_kernel_spmd(nc, in_maps, core_ids=list(range(NCORES)))
    full = np.concatenate(
        [r["out"].T.reshape(bs, S, El) for r in res.results], axis=0
    )
    if b_val != 0.0:
        full = full + b_val
    return np.ascontiguousarray(full.astype(np.float32))


# revision 69
# speedup vs baseline: 1.1184x; 1.1184x over previous
"""Trainium2 Bass kernel for char-CNN: 5-tap conv along word_length + max-pool.

Reference computation (per (batch, sentence) word, shapes B=64 S=256 W=20 E=128):
    y[w, e] = sum_{kh=0..4} x[w + kh - 2, e] * conv_w[kh]     (zero padded)
    out[e]  = max_w y[w, e] + conv_b

Strategy:
  - Data-parallel over 8 NeuronCores: 8 batches (2048 words) per core.
  - Host pre-arranges each core's shard to z[(j w)=120, group=342, e=128]
    (groups of J=6 words, last group zero-padded) so every DMA descriptor
    is a multi-KiB contiguous run per partition — full HBM bandwidth.
  - The conv is a banded 20x20 matrix applied per word, done on TensorE:
    stationary lhsT = x6 [K=120 (6 words x 20 w_in), M=128 (e)], moving
    rhs = block-diagonal A [120, 120] -> PSUM [128 (e), 120 (6w x 20 w_out)].
    fp16 operands (1 cycle/row on PE; fp32 would be 4).
  - Max over w_out is a free-dim reduce on VectorE straight out of PSUM:
    [128, (groups, 20)] -> [128, groups*6] into a persistent [128, NW]
    maxima tile; one DMA out at the end (host transposes back).
  - Input DMAs are spread across the SP-HWDGE / ACT-HWDGE / SWDGE rings so
    the 16 SDMA engines always have in-flight work (one FIFO ring alone
    leaves completion-latency bubbles).  The SWDGE (gpsimd) ring casts
    f32 -> f16 in the DMA datapath; HWDGE rings land f32 and ScalarE casts.
"""

from contextlib import ExitStack

import numpy as np

import concourse.bass as bass
import concourse.mybir as mybir
import concourse.tile as tile
from concourse import bacc

W = 20  # word length
E = 128  # embedding dim
KH = 5  # conv taps
PAD = 2
J = 6  # words per matmul group (6 * 20 = 120 <= 128 partitions)
KP = J * W  # contraction size / partitions used (120)
CG = 16  # groups per compute sub-chunk (4 PSUM banks)
NCORES = 8
BANK = 512  # PSUM bank size in f32 elements


def build_conv_matrix(conv_w: np.ndarray) -> np.ndarray:
    """[KP, KP] conv matrix, output columns ordered w_out-major:
    A[j*W + wi, wo*J + j] = conv_w[wi - wo + 2].  The w-major column
    order makes the PSUM output planar so every max level on VectorE is
    a contiguous step-1 f16 tensor_max (2x mode)."""
    wv = np.asarray(conv_w, np.float32).reshape(-1)
    assert wv.shape == (KH,)
    a = np.zeros((KP, KP), np.float32)
    for j in range(J):
        for wo in range(W):
            for kh in range(KH):
                wi = wo + kh - PAD
                if 0 <= wi < W:
                    a[j * W + wi, wo * J + j] = wv[kh]
    return a.astype(np.float16)


def pack_input(x_core: np.ndarray, ng: int) -> np.ndarray:
    """[nw, W, E] f32 -> [KP, ng, E] f16 partition-major, zero-padded to
    ng*J words. The fp16 cast is the same one the kernel's compute path
    uses (TensorE consumes fp16); doing it host-side halves HBM traffic."""
    nw = x_core.shape[0]
    xp = np.zeros((ng * J, W, E), np.float16)
    xp[:nw] = x_core.astype(np.float16)
    # (g j) w e -> (j w) g e
    return np.ascontiguousarray(
        xp.reshape(ng, J, W, E).transpose(1, 2, 0, 3).reshape(KP, ng, E)
    )


def chunk_plan(ng: int, big: int = 64) -> list[int]:
    """Descending chunk sizes: big early (fewer ring bubbles while the
    stream is deep), small at the end (short pipeline tail)."""
    sizes = []
    rem = ng
    for sz, keep in ((64, 96), (32, 48), (16, 24), (8, 8)):
        if sz > big:
            continue
        while rem >= max(sz, keep):
            sizes.append(sz)
            rem -= sz
    if rem:
        sizes.append(rem)
    return sizes


def build_nc(
    nw: int,
    dma_rings: tuple[str, ...] = ("gpsimd",),
    bufs: int = 24,
    first_ring: str | None = None,
    big_chunk: int = 16,
    cg: int = 16,
    psum_bufs: int = 2,
    d_every: int = 12,
    deep_spool: int = 8,
) -> bass.Bass:
    """Build the per-core Bass graph. nw = real words per core.

    dma_rings: which descriptor rings carry the input stream, round-robin
    per chunk. 'gpsimd' (SWDGE) casts f32->f16 during the DMA; HWDGE rings
    ('sync'/'scalar') land f32 and ScalarE casts to f16.
    """
    f32 = mybir.dt.float32
    f16 = mybir.dt.float16
    ng = (nw + J - 1) // J  # padded group count
    nwp = ng * J  # padded word count

    nc = bacc.Bacc()
    z_ext = nc.declare_dram_parameter("z", [KP, ng, E], f16, isOutput=False)
    a_ext = nc.declare_dram_parameter("a", [KP, KP], f16, isOutput=False)
    out_ext = nc.declare_dram_parameter("out", [E, nw], f32, isOutput=True)

    engines = {
        "sync": nc.sync,
        "scalar": nc.scalar,
        "gpsimd": nc.gpsimd,
    }

    with ExitStack() as ctx:
        tc = ctx.enter_context(tile.TileContext(nc))
        const = ctx.enter_context(tc.tile_pool(name="const", bufs=1))
        hpool = ctx.enter_context(tc.tile_pool(name="xh", bufs=bufs))
        opool = ctx.enter_context(tc.tile_pool(name="o", bufs=1))
        spool = ctx.enter_context(tc.tile_pool(name="ys", bufs=deep_spool))
        t1pool = ctx.enter_context(tc.tile_pool(name="t1", bufs=deep_spool))
        u2pool = ctx.enter_context(tc.tile_pool(name="u2", bufs=deep_spool))
        u3pool = ctx.enter_context(tc.tile_pool(name="u3", bufs=deep_spool))
        u4pool = ctx.enter_context(tc.tile_pool(name="u4", bufs=deep_spool))
        pspool = ctx.enter_context(
            tc.tile_pool(name="ps", bufs=psum_bufs, space="PSUM")
        )
        ps_banks = (cg + 3) // 4  # PSUM banks per compute sub-chunk

        a_t = const.tile([KP, KP], f16)
        nc.sync.dma_start(out=a_t[:, :], in_=a_ext[:, :])
        maxt = opool.tile([E, nwp], f32)

        HW = W // 2  # 10

        def do_matmuls(xh, coff, sn):
            ps = pspool.tile([E, ps_banks * BANK], f32, tag="ps")
            for g in range(sn):
                col = (g // 4) * BANK + (g % 4) * KP
                nc.tensor.matmul(
                    ps[:, col : col + KP],
                    lhsT=xh[:, coff + g * E : coff + (g + 1) * E],
                    rhs=a_t[:, :],
                    start=True,
                    stop=True,
                )
            return ps

        def psum_view(ps, sn):
            """[E, nbank, c, W] view of sn (multiple of 4) groups."""
            nbank = sn // 4
            return (
                ps[:, 0 : nbank * BANK]
                .rearrange("p (k x) -> p k x", k=nbank)[:, :, 0 : 4 * J * W]
                .rearrange("p k (c w) -> p k c w", w=W)
            )

        def stage12_act(xh, coff, sg0, sn):
            """ACT parks the 20-block in SBUF as f16 in a TRANSPOSED planar
            layout s[w][word] (free: ACT is 1x regardless of write stride).
            Every max level is then a contiguous step-1 f16 tensor_max in
            DVE 2x mode: 20 -> 10 -> 5 -> (2,2,1) -> 1.  sn % 4 == 0."""
            ps = do_matmuls(xh, coff, sn)
            # layout-preserving copy: psum cols are already (g, w, j)
            pin = ps[:, 0 : (sn // 4) * BANK].rearrange(
                "p (k x) -> p k x", k=sn // 4
            )[:, :, 0 : 4 * J * W]
            s = spool.tile([E, cg * J * W], f16, tag="ys")
            sv = s[:, 0 : sn * J * W].rearrange(
                "p (k x) -> p k x", k=sn // 4
            )
            nc.scalar.copy(sv, pin)
            # max tree: every level is max over the w axis of (g, w, j)
            # blocks — contiguous J-element runs, f16 2x mode throughout
            def tview(tile, nw_, w0, w1):
                return tile[:, 0 : sn * nw_ * J].rearrange(
                    "p (g w j) -> p g w j", g=sn, j=J
                )[:, :, w0:w1, :]

            u1 = t1pool.tile([E, cg * J * HW], f16, tag="t1")
            nc.vector.tensor_max(
                u1[:, 0 : sn * HW * J], tview(s, W, 0, HW), tview(s, W, HW, W)
            )
            u2 = u2pool.tile([E, cg * J * 5], f16, tag="u2")
            nc.vector.tensor_max(
                u2[:, 0 : sn * 5 * J], tview(u1, HW, 0, 5), tview(u1, HW, 5, 10)
            )
            u3 = u3pool.tile([E, cg * J * 2], f16, tag="u3")
            nc.vector.tensor_max(
                u3[:, 0 : sn * 2 * J], tview(u2, 5, 0, 2), tview(u2, 5, 2, 4)
            )
            u4 = u4pool.tile([E, cg * J], f16, tag="u4")
            nc.vector.tensor_max(
                u4[:, 0 : sn * J], tview(u3, 2, 0, 1), tview(u3, 2, 1, 2)
            )
            # final merge with the leftover 5th w-plane, f32 out to maxt
            nc.vector.tensor_max(
                maxt[:, sg0 * J : (sg0 + sn) * J].rearrange(
                    "p (g w j) -> p g w j", g=sn, j=J
                ),
                tview(u4, 1, 0, 1),
                tview(u2, 5, 4, 5),
            )
            return ("a", None, sg0, sn)

        def stage12_direct(xh, coff, sg0, sn):
            ps = do_matmuls(xh, coff, sn)
            return ("d", ps, sg0, sn)

        def stage3(kind, t, sg0, sn):
            if kind == "a":
                return  # the 'a' pipeline already wrote maxt
            # direct: reduce over w straight out of PSUM. Columns are
            # (g, w, j) per bank, so put w innermost in the AP.
            wcur = sg0 * J
            for b in range((sn + 3) // 4):
                gb = min(4, sn - 4 * b)
                pv = t[:, BANK * b : BANK * b + gb * J * W].rearrange(
                    "p (g w j) -> p g j w", g=gb, j=J
                )
                out_v = maxt[:, wcur : wcur + gb * J].rearrange(
                    "p (g j) -> p g j", g=gb
                )
                nc.vector.reduce_max(out_v, pv, axis=mybir.AxisListType.X)
                wcur += gb * J

        g0 = 0
        if first_ring is not None:
            sizes = [16] + chunk_plan(ng - 16, big_chunk)
            rings = [first_ring] + [
                dma_rings[i % len(dma_rings)] for i in range(len(sizes) - 1)
            ]
        else:
            if ng > 32:
                # two small warm-up chunks so compute starts sooner
                sizes = [8, 8] + chunk_plan(ng - 16, big_chunk)
            else:
                sizes = chunk_plan(ng, big_chunk)
            rings = [dma_rings[i % len(dma_rings)] for i in range(len(sizes))]
        max_gn = max(sizes)

        # Phase A: the whole input stream is issued up front (bufs covers
        # every chunk) so no compute op can head-of-line-block a DMA
        # trigger on the gpsimd FIFO.
        subs = []
        for ring, gn in enumerate(sizes):
            eng_name = rings[ring]
            src = z_ext[:, g0 : g0 + gn, :].rearrange("p g e -> p (g e)")
            xh = hpool.tile([KP, max_gn * E], f16, tag="xh")
            engines[eng_name].dma_start(out=xh[:, 0 : gn * E], in_=src)
            for s0 in range(0, gn, cg):
                sn = min(cg, gn - s0)
                subs.append((xh, s0 * E, g0 + s0, sn))
            g0 += gn

        # Phase B: compute pipeline. stage3 follows its stage12 directly:
        # its dependency is the immediately preceding same-engine op (the
        # DVE tensor_max for ACT-path subs, the PE matmuls for direct).
        w_flushed = 0

        def flush_out(upto_words):
            nonlocal w_flushed
            hi = min(upto_words, nw)
            if hi - w_flushed >= 192 or (hi >= nw and hi > w_flushed):
                nc.sync.dma_start(
                    out=out_ext[:, w_flushed:hi], in_=maxt[:, w_flushed:hi]
                )
                w_flushed = hi

        for idx, sub in enumerate(subs):
            _, _, _, sn = sub
            if sn % 4 == 0 and (d_every == 0 or idx % d_every != d_every - 1):
                kind, t, sg0, sn = stage12_act(*sub)
            else:
                kind, t, sg0, sn = stage12_direct(*sub)
            stage3(kind, t, sg0, sn)
            flush_out(sg0 * J + sn * J)
    nc.finalize()
    return nc


def kernel(embedded_char, conv_w, conv_b):
    from concourse.bass_utils import run_bass_kernel_spmd

    x = np.asarray(embedded_char, np.float32)
    b_val = float(np.asarray(conv_b, np.float32).reshape(-1)[0])
    B, S, Wl, El = x.shape
    assert (Wl, El) == (W, E)
    bs = B // NCORES
    nw = bs * S
    ng = (nw + J - 1) // J
    a16 = build_conv_matrix(conv_w)

    nc = build_nc(nw)
    in_maps = [
        {
            "z": pack_input(x[i * bs : (i + 1) * bs].reshape(nw, Wl, El), ng),
            "a": a16,
        }
        for i in range(NCORES)
    ]
    res = run_bass_kernel_spmd(nc, in_maps, core_ids=list(range(NCORES)))
    full = np.concatenate(
        [r["out"].T.reshape(bs, S, El) for r in res.results], axis=0
    )
    if b_val != 0.0:
        full = full + b_val
    return np.ascontiguousarray(full.astype(np.float32))
